# revision 12
# baseline (speedup 1.0000x reference)
"""CorefHead Trainium2 kernel.

Reference computation (B=64, S=512, H=1024, HID=512):
  emb_a = span_mean(bert, offsets[:,0:2])   # [B,H]
  emb_b = span_mean(bert, offsets[:,2:4])   # [B,H]
  emb_p = bert[b, offsets[:,4]]             # [B,H]
  x = concat([emb_a, emb_b, emb_p], -1)     # [B,3H]
  h = leaky_relu(batchnorm_eval(x @ W1 + b1), 0.01)
  out = h @ W2 + b2                         # [B,3]

Strategy: pure data parallel, batch sharded 8 ways (8 batches/core).
The kernel is HBM-stream-bound (~0.39 B/ns/core, ~0.2 B/ns per HWDGE
ring, with all 8 cores streaming), so the schedule is built around the
arrival order of one continuous stream:
  - Tiny consts ride the FRONT of ring1 (HWDGE, ~9.5us arrival; the
    gpsimd SWDGE path takes >14us for the same bytes).
  - bert rows ship next on both rings, balanced by bytes: fp8 e4m3 for
    spans >= 16 rows (quantization noise averages out over the span),
    bf16 for shorter spans + pron rows. Rows are packed partition-major
    with MW=24 mask columns (3 embeddings x 8 batches) per 128-row
    chunk, mask region padded to 32 so the chunk width (1056) is
    16-divisible (DoubleRow AP constraint).
  - mm1 uses fp8 DoubleRow: one matmul contracts a PAIR of 128-row
    chunks ([128,2,24] stationary, [128,2,512] moving) at 2x column
    rate (~108 ns/chunk warm), so mm1 tracks the stream. Chunk pairs
    are processed in estimated arrival order.
  - W1 ships strictly AFTER bert, in mm2's hc-major consumption order
    (span rows fp8 e3m4 x16 on ring0, pron rows bf16 on ring1 -- pron
    x-values are ~12x larger, so pron W1 stays bf16 for accuracy),
    with small tail pieces so the last-arriving bytes feed the
    shortest remaining compute.
  - PE clock pre-warm (HAM) on memset junk fed by the Vector engine;
    the BN-bias matmul runs inside the warmup window (consts are
    already resident), so the PE never idles between warmup and mm1.
  - Tail: lrelu split in halves overlaps ACT with the PE transposes;
    b2 folds into the PSUM->SBUF output copy via ACT per-partition
    bias. Host gathers per-core [3, 8] outputs and undoes the batch
    permutation.
"""

import numpy as np

B, S, H = 64, 512, 1024
HID = 512
EPS = 1e-5
NCORES = 8
BPC = B // NCORES  # batches per core
NMC = 3 * BPC      # mask columns: (embedding e, batch slot b) -> e*BPC + b
MW = NMC           # mask width (cols 0:NMC of each chunk)
PADW = 32          # mask region padded so CW % 16 == 0 (DoubleRow AP rule)
CW = PADW + H      # chunk width: mask region + bert row

# Span rows >= T ship as fp8 e4m3 (DoubleRow-capable); shorter spans
# (and pron rows) ship bf16.
T_FP8 = 16
W1_SCALE = 16.0    # prescale for W1 span rows in fp8 e3m4
N_WARM = 8         # dummy matmuls to pre-warm the PE clock
RING_RATE = 0.195  # per-ring stream rate, B/ns (for arrival ordering)

TRACE = False
LAST_RESULT = None

_PROGRAM_CACHE: dict = {}

# W1 piece layout (consumption-ordered, small tail pieces). Span pieces
# slice the [128, 16, HID] e3m4 tensor (k = hc*2+e); pron pieces slice
# the [128, 8, HID] bf16 tensor (k = hc). Piece count is kept low: the
# shared 10-semaphore DMA pool serializes issue across rings when
# descriptors are too numerous.
W1S_PIECES = [(0, 8), (8, 4), (12, 2), (14, 1), (15, 1)]
W1P_PIECES = [(0, 4), (4, 2), (6, 1), (7, 1)]
E3_PIECE = 4       # chunks per e3 DMA piece (multiple of 2 for DoubleRow)

# cstAB layout (single [128, 560] bf16 const buffer; >=512B lines so the
# SDMA engines run at line rate): [0:24]=ident24, [24:36]=W2 (4x3),
# [36]=b2 (rows 0:3), row0 [37:549]=bn bias, row0 [549:557]=ones
CAB_W = 560


def _build_program(nch8: int, nchb: int, bp: int):
    import concourse.bacc as bacc
    import concourse.tile as tile
    import concourse.mybir as mybir
    from concourse.bass import MemorySpace

    f32 = mybir.dt.float32
    bf = mybir.dt.bfloat16
    e4 = mybir.dt.float8e4
    w8 = mybir.dt.float8e3
    DR = mybir.MatmulPerfMode.DoubleRow

    nc = bacc.Bacc("TRN2", target_bir_lowering=False, debug=False,
                   num_devices=NCORES)

    HC = H // 128       # 8

    e3_d = nc.dram_tensor("e3buf", [128, nch8, CW], e4,
                          kind="ExternalInput").ap()
    bf_d = nc.dram_tensor("bfbuf", [bp, nchb, CW], bf,
                          kind="ExternalInput").ap()
    w1s_d = nc.dram_tensor("w1S", [128, 16, HID], w8,
                           kind="ExternalInput").ap()
    w1p_d = nc.dram_tensor("w1P", [128, 8, HID], bf,
                           kind="ExternalInput").ap()
    cstAB_d = nc.dram_tensor("cstAB", [128, CAB_W], bf,
                             kind="ExternalInput").ap()
    # sfac replicated 128x along the free dim so the DMA moves 512B lines
    sfac_d = nc.dram_tensor("sfac", [NMC, 128], f32, kind="ExternalInput").ap()
    out_d = nc.dram_tensor("out", [3, BPC], f32, kind="ExternalOutput").ap()

    with tile.TileContext(nc) as tc:
        with (
            tc.tile_pool(name="data", bufs=1) as data,
            tc.tile_pool(name="work", bufs=1) as work,
            tc.tile_pool(name="psum_x", bufs=1, space=MemorySpace.PSUM) as psx,
            tc.tile_pool(name="psum_t", bufs=4, space=MemorySpace.PSUM) as pst,
            tc.tile_pool(name="psum_h", bufs=1, space=MemorySpace.PSUM) as psh,
        ):
            # --- PE pre-warm on memset junk (fed by the otherwise-idle
            # Vector engine, whose preamble retires earliest).
            junk = work.tile([128, 512], bf, tag="junk")
            nc.vector.memset(junk, 0.0)
            scr = psh.tile([24, 512], f32, tag="scratch")
            for _ in range(N_WARM):
                nc.tensor.matmul(scr, junk[:, 0:24], junk, start=True,
                                 stop=True)

            # --- DMA issue. ring0 = sync, ring1 = scalar (HWDGE).
            # ring1 leads with the tiny consts, then the bf16 buffer;
            # e3 pieces are balanced across rings by bytes; W1 pieces
            # follow strictly after bert, in consumption order.
            ring_eng = [nc.sync, nc.scalar]
            ring_bytes = [0.0, 0.0]

            cstAB_t = data.tile([128, CAB_W], bf, tag="cstAB")
            nc.sync.dma_start(out=cstAB_t, in_=cstAB_d)
            ring_bytes[0] += 128 * CAB_W * 2
            sfac_t = data.tile([NMC, 128], f32, tag="sfac")
            nc.scalar.dma_start(out=sfac_t, in_=sfac_d)
            ring_bytes[1] += NMC * 128 * 4

            bf_t = data.tile([bp, nchb, CW], bf, tag="bfp")
            nc.scalar.dma_start(out=bf_t, in_=bf_d)
            ring_bytes[1] += bp * nchb * CW * 2

            # e3 pieces: E3_PIECE chunks each (chunk pairs stay within a
            # piece for DoubleRow), assigned greedily to the lighter
            # ring; remember arrival estimates.
            e3_ts = []
            for i, c0 in enumerate(range(0, nch8, E3_PIECE)):
                pc = min(E3_PIECE, nch8 - c0)
                t = data.tile([128, pc, CW], e4, tag=f"e3p{i}",
                              name=f"e3p{i}")
                r = int(np.argmin(ring_bytes))
                ring_eng[r].dma_start(out=t, in_=e3_d[:, c0:c0 + pc, :])
                ring_bytes[r] += pc * 128 * CW
                e3_ts.append((ring_bytes[r] / RING_RATE, pc, t))

            w1s_ts = []
            for i, (k0, kn) in enumerate(W1S_PIECES):
                t = data.tile([128, kn, HID], w8, tag=f"w1s{i}",
                              name=f"w1s{i}")
                ring_eng[0].dma_start(out=t, in_=w1s_d[:, k0:k0 + kn, :])
                w1s_ts.append((k0, kn, t))
            w1p_ts = []
            for i, (k0, kn) in enumerate(W1P_PIECES):
                t = data.tile([128, kn, HID], bf, tag=f"w1p{i}",
                              name=f"w1p{i}")
                ring_eng[1].dma_start(out=t, in_=w1p_d[:, k0:k0 + kn, :])
                w1p_ts.append((k0, kn, t))

            def w1_slice(hc, e):
                if e < 2:
                    k = hc * 2 + e
                    for k0, kn, t in w1s_ts:
                        if k0 <= k < k0 + kn:
                            return t[:, k - k0, :]
                for k0, kn, t in w1p_ts:
                    if k0 <= hc < k0 + kn:
                        return t[:, hc - k0, :]
                raise AssertionError

            # ACT table loads for Identity/Lrelu
            jact = work.tile([128, 32], bf, tag="jact")
            nc.scalar.activation(jact, junk[:, 0:32],
                                 mybir.ActivationFunctionType.Identity,
                                 scale=2.0)
            nc.scalar.activation(jact, junk[:, 0:32],
                                 mybir.ActivationFunctionType.Lrelu,
                                 alpha=0.01)

            # --- BN bias into ph inside the warmup window (consts are
            # on ring1's head, resident by ~9.5us)
            ph = psh.tile([BPC, HID], f32, tag="ph")
            nc.tensor.matmul(ph, cstAB_t[0:1, 549:557], cstAB_t[0:1, 37:549],
                             start=True, stop=False)

            # --- mm1: x[24, 1024] += mask.T @ bert over chunks.
            # bf16 chunks first (earliest arrival, run during the clock
            # ramp), then fp8 chunk pairs via DoubleRow in estimated
            # arrival order.
            px0 = psx.tile([NMC, 512], f32, tag="px0")
            px1 = psx.tile([NMC, 512], f32, tag="px1")

            order = sorted(range(len(e3_ts)), key=lambda i: e3_ts[i][0])

            def mm1_pairs(j, first, last_any):
                _, pc, t = e3_ts[order[j]]
                for lp in range(0, pc, 2):
                    m = t[:, lp:lp + 2, 0:MW]
                    nc.tensor.matmul(px0, m, t[:, lp:lp + 2, PADW:PADW + 512],
                                     start=first and lp == 0, stop=False,
                                     perf_mode=DR)
                    nc.tensor.matmul(px1, m, t[:, lp:lp + 2, PADW + 512:CW],
                                     start=first and lp == 0,
                                     stop=last_any and lp + 2 >= pc,
                                     perf_mode=DR)

            head = min(2, len(order))
            for j in range(head):
                mm1_pairs(j, j == 0, False)
            for c in range(nchb):
                m = bf_t[:, c, 0:MW]
                nc.tensor.matmul(px0, m, bf_t[:, c, PADW:PADW + 512],
                                 start=False, stop=False)
                nc.tensor.matmul(px1, m, bf_t[:, c, PADW + 512:CW],
                                 start=False, stop=False)
            for j in range(head, len(order)):
                mm1_pairs(j, False, j == len(order) - 1)

            # --- x: scale by 1/len (fp32) + cast bf16, split ACT/DVE
            # --- x: scale by 1/len (fp32) + cast bf16 in quarter
            # slices (ACT on px0 halves, DVE on px1 halves) so the PE
            # transposes start as soon as their slice lands
            xsb = work.tile([NMC, H], bf, tag="xsb")
            ident24 = cstAB_t[0:NMC, 0:NMC]
            xT = work.tile([128, HC, NMC], bf, tag="xT")
            for half in range(2):
                s0 = half * 256
                nc.scalar.activation(xsb[:, s0:s0 + 256], px0[:, s0:s0 + 256],
                                     mybir.ActivationFunctionType.Identity,
                                     scale=sfac_t[:, 0:1])
                nc.vector.tensor_scalar_mul(xsb[:, 512 + s0:768 + s0],
                                            px1[:, s0:s0 + 256],
                                            sfac_t[:, 0:1])
            for hc in range(HC):
                pT = pst.tile([128, NMC], bf, tag="pT")
                nc.tensor.transpose(pT, xsb[:, hc * 128:(hc + 1) * 128],
                                    ident24)
                nc.vector.tensor_copy(xT[:, hc, :], pT)
            for hc in range(HC):
                for e in range(3):
                    nc.tensor.matmul(
                        ph, xT[:, hc, e * BPC:(e + 1) * BPC], w1_slice(hc, e),
                        start=False, stop=(hc == HC - 1 and e == 2),
                    )

            # --- LeakyReLU on ACT in halves (overlaps PE transposes)
            y = work.tile([BPC, HID], bf, tag="y")
            ident8 = cstAB_t[0:BPC, 0:BPC]
            yT_ps = [pst.tile([128, BPC], bf, tag="pT", name=f"yTp{mc}")
                     for mc in range(4)]
            yT_sb = [work.tile([128, BPC], bf, tag=f"yTs{mc}", name=f"yTs{mc}")
                     for mc in range(4)]
            for half in range(2):
                nc.scalar.activation(y[:, half * 256:(half + 1) * 256],
                                     ph[:, half * 256:(half + 1) * 256],
                                     mybir.ActivationFunctionType.Lrelu,
                                     alpha=0.01)
                for mc in (2 * half, 2 * half + 1):
                    nc.tensor.transpose(
                        yT_ps[mc], y[:, mc * 128:(mc + 1) * 128], ident8)
                    nc.vector.tensor_copy(yT_sb[mc], yT_ps[mc])

            # --- mm3: out[3, 8] = W2.T @ y.T; b2 folds into the copy
            oT = psx.tile([3, BPC], f32, tag="px0")
            for mc in range(4):
                nc.tensor.matmul(oT, cstAB_t[:, 24 + 3 * mc:27 + 3 * mc],
                                 yT_sb[mc], start=(mc == 0), stop=(mc == 3))
            o_sb = work.tile([3, BPC], f32, tag="osb")
            nc.scalar.activation(o_sb, oT,
                                 mybir.ActivationFunctionType.Identity,
                                 bias=cstAB_t[0:3, 36:37])
            nc.sync.dma_start(out=out_d, in_=o_sb)

    nc.compile()
    return nc


def _pack_rows(rows, masks, nch, np_dt, part=128):
    """rows: [N, H] fp32, masks: [N, NMC] fp32 -> [part, nch, CW] np_dt,
    partition-major (packed position i -> (p=i%part, c=i//part))."""
    N = rows.shape[0]
    buf = np.zeros((nch * part, CW), dtype=np.float32)
    if N:
        buf[:N, :MW] = masks
        buf[:N, PADW:] = rows
    return np.ascontiguousarray(
        buf.reshape(nch, part, CW).transpose(1, 0, 2)).astype(np_dt)


def _prep_core_inputs(bert, offs, w1_bufs, cstAB, batch_idx,
                      nch8, nchb, bp):
    import ml_dtypes
    bf16 = ml_dtypes.bfloat16
    e4 = ml_dtypes.float8_e4m3
    f8_rows, f8_masks = [], []
    b16_rows, b16_masks = [], []
    sfac = np.ones((NMC, 1), dtype=np.float32)
    for slot, gb in enumerate(batch_idx):
        a0, a1, b0, b1_, p = (int(v) for v in offs[gb])
        spans = [(a0, a1, 0), (b0, b1_, 1)]
        long_spans = [s for s in spans if s[1] - s[0] + 1 >= T_FP8]
        short_spans = [s for s in spans if s[1] - s[0] + 1 < T_FP8]
        for (lo, hi, e) in spans:
            wsc = W1_SCALE if e < 2 else 1.0
            sfac[e * BPC + slot, 0] = 1.0 / ((hi - lo + 1) * wsc)
        if long_spans:
            lo = min(s[0] for s in long_spans)
            hi = max(s[1] for s in long_spans)
            pos = np.arange(lo, hi + 1)
            keep = np.zeros(len(pos), dtype=bool)
            m = np.zeros((len(pos), NMC), dtype=np.float32)
            for (s0, s1, e) in long_spans:
                sel = (pos >= s0) & (pos <= s1)
                keep |= sel
                m[sel, e * BPC + slot] = 1.0
            f8_rows.append(bert[gb, pos[keep]])
            f8_masks.append(m[keep])
        want = {}
        for (s0, s1, e) in short_spans:
            for r in range(s0, s1 + 1):
                want.setdefault(r, []).append(e)
        want.setdefault(p, []).append(2)
        if want:
            rs = sorted(want)
            m = np.zeros((len(rs), NMC), dtype=np.float32)
            for i, r in enumerate(rs):
                for e in want[r]:
                    m[i, e * BPC + slot] = 1.0
            b16_rows.append(bert[gb, rs])
            b16_masks.append(m)

    def cat(parts, w):
        return (np.concatenate(parts, axis=0) if parts
                else np.zeros((0, w), dtype=np.float32))

    in_map = {
        "e3buf": _pack_rows(cat(f8_rows, H), cat(f8_masks, NMC), nch8, e4),
        "bfbuf": _pack_rows(cat(b16_rows, H), cat(b16_masks, NMC), nchb, bf16,
                            part=bp),
        "cstAB": cstAB,
        "sfac": np.repeat(sfac, 128, axis=1),
    }
    in_map.update(w1_bufs)
    return in_map


def _row_counts(offs):
    """Per-batch (fp8 rows, bf16 rows) under the T_FP8 split."""
    n8 = np.zeros(B, dtype=np.int64)
    nb = np.zeros(B, dtype=np.int64)
    for gb in range(B):
        a0, a1, b0, b1_, p = (int(v) for v in offs[gb])
        spans = [(a0, a1), (b0, b1_)]
        longs = [s for s in spans if s[1] - s[0] + 1 >= T_FP8]
        shorts = [s for s in spans if s[1] - s[0] + 1 < T_FP8]
        if longs:
            lo = min(s[0] for s in longs)
            hi = max(s[1] for s in longs)
            keep = np.zeros(hi - lo + 1, dtype=bool)
            for (s0, s1) in longs:
                keep[s0 - lo:s1 - lo + 1] = True
            n8[gb] = keep.sum()
        rows = set()
        for (s0, s1) in shorts:
            rows.update(range(s0, s1 + 1))
        rows.add(p)
        nb[gb] = len(rows)
    return n8, nb


def kernel(bert_outputs, offsets, W1, b1, gamma, beta, running_mean,
           running_var, W2, b2):
    import ml_dtypes
    bf16 = ml_dtypes.bfloat16
    e3 = ml_dtypes.float8_e3m4

    bert = np.ascontiguousarray(np.asarray(bert_outputs, dtype=np.float32))
    offs = np.asarray(offsets).astype(np.int64)
    W1 = np.asarray(W1, dtype=np.float32)
    b1 = np.asarray(b1, dtype=np.float32)
    gamma = np.asarray(gamma, dtype=np.float32)
    beta = np.asarray(beta, dtype=np.float32)
    rm = np.asarray(running_mean, dtype=np.float32)
    rv = np.asarray(running_var, dtype=np.float32)
    W2 = np.asarray(W2, dtype=np.float32)
    b2 = np.asarray(b2, dtype=np.float32)

    # Fold BN eval stats: bn(xW1 + b1) = x(W1*s) + ((b1 - mean)*s + beta)
    s = gamma / np.sqrt(rv + EPS)
    bias = (b1 - rm) * s + beta
    W1s = W1 * s[None, :]
    w1ehc = W1s.reshape(3, 8, 128, HID)  # [e, hc, p, n]
    w1_bufs = {
        # [p, hc*2+e, n] for span embeddings e in {0,1}, fp8 e3m4 x16
        "w1S": np.ascontiguousarray(
            w1ehc[:2].transpose(2, 1, 0, 3).reshape(128, 16, HID)
            * W1_SCALE).astype(e3),
        # [p, hc, n] for the pron embedding, bf16
        "w1P": np.ascontiguousarray(
            w1ehc[2].transpose(1, 0, 2)).astype(bf16),
    }

    cstAB = np.zeros((128, CAB_W), dtype=np.float32)
    cstAB[:NMC, :NMC] = np.eye(NMC)
    cstAB[:, 24:36] = W2.reshape(4, 128, 3).transpose(1, 0, 2).reshape(128, 12)
    cstAB[0:3, 36] = b2
    cstAB[0, 37:549] = bias
    cstAB[0, 549:557] = 1.0
    cstAB = cstAB.astype(bf16)

    # Greedy-balance batches across cores by shipped bytes (fp8 row =
    # CW bytes, bf16 row = 2*CW), capped at BPC batches per core
    n8, nb = _row_counts(offs)
    cost = n8 + 2 * nb
    order = np.argsort(-cost, kind="stable")
    core_rows8 = np.zeros(NCORES, dtype=np.int64)
    core_rowsb = np.zeros(NCORES, dtype=np.int64)
    core_batches = [[] for _ in range(NCORES)]
    for gb in order:
        load = core_rows8 + 2 * core_rowsb
        load[np.array([len(cb) >= BPC for cb in core_batches])] = 1 << 40
        c = int(np.argmin(load))
        core_batches[c].append(int(gb))
        core_rows8[c] += n8[gb]
        core_rowsb[c] += nb[gb]
    nch8 = max(2, int((core_rows8.max() + 127) // 128))
    nch8 += nch8 % 2  # even chunk count for DoubleRow pairs
    maxb = int(core_rowsb.max())
    bp = 128  # full-partition DMA lines; narrow transfers crawl
    nchb = max(1, (maxb + 127) // 128)

    key = (nch8, nchb, bp)
    if key not in _PROGRAM_CACHE:
        _PROGRAM_CACHE[key] = _build_program(nch8, nchb, bp)
    nc = _PROGRAM_CACHE[key]

    in_maps = [
        _prep_core_inputs(bert, offs, w1_bufs, cstAB, core_batches[c],
                          nch8, nchb, bp)
        for c in range(NCORES)
    ]

    from concourse import bass_utils
    kwargs = {}
    if TRACE:
        kwargs = {"trace": True, "trace_cores": list(range(NCORES))}
    res = bass_utils.run_bass_kernel_spmd(nc, in_maps,
                                          core_ids=list(range(NCORES)),
                                          **kwargs)
    global LAST_RESULT
    LAST_RESULT = res

    out = np.empty((B, 3), dtype=np.float32)
    for c in range(NCORES):
        out[core_batches[c]] = res.results[c]["out"].T
    return out


# revision 13
# speedup vs baseline: 1.0341x; 1.0341x over previous
"""CorefHead Trainium2 kernel.

Reference computation (B=64, S=512, H=1024, HID=512):
  emb_a = span_mean(bert, offsets[:,0:2])   # [B,H]
  emb_b = span_mean(bert, offsets[:,2:4])   # [B,H]
  emb_p = bert[b, offsets[:,4]]             # [B,H]
  x = concat([emb_a, emb_b, emb_p], -1)     # [B,3H]
  h = leaky_relu(batchnorm_eval(x @ W1 + b1), 0.01)
  out = h @ W2 + b2                         # [B,3]

Strategy: pure data parallel, batch sharded 8 ways (8 batches/core).
The kernel is HBM-stream-bound (~0.39 B/ns/core, ~0.2 B/ns per HWDGE
ring, with all 8 cores streaming), so the schedule is built around the
arrival order of one continuous stream:
  - Tiny consts ride the FRONT of ring1 (HWDGE, ~9.5us arrival; the
    gpsimd SWDGE path takes >14us for the same bytes).
  - bert rows ship next on both rings, balanced by bytes: fp8 e4m3 for
    spans >= 16 rows (quantization noise averages out over the span),
    bf16 for shorter spans + pron rows. Rows are packed partition-major
    with MW=24 mask columns (3 embeddings x 8 batches) per 128-row
    chunk, mask region padded to 32 so the chunk width (1056) is
    16-divisible (DoubleRow AP constraint).
  - mm1 uses fp8 DoubleRow: one matmul contracts a PAIR of 128-row
    chunks ([128,2,24] stationary, [128,2,512] moving) at 2x column
    rate (~108 ns/chunk warm), so mm1 tracks the stream. Chunk pairs
    are processed in estimated arrival order.
  - W1 ships strictly AFTER bert, in mm2's hc-major consumption order
    (span rows fp8 e3m4 x16 on ring0, pron rows bf16 on ring1 -- pron
    x-values are ~12x larger, so pron W1 stays bf16 for accuracy),
    with small tail pieces so the last-arriving bytes feed the
    shortest remaining compute.
  - PE clock pre-warm (HAM) on memset junk fed by the Vector engine;
    the BN-bias matmul runs inside the warmup window (consts are
    already resident), so the PE never idles between warmup and mm1.
  - Tail: lrelu split in halves overlaps ACT with the PE transposes;
    b2 folds into the PSUM->SBUF output copy via ACT per-partition
    bias. Host gathers per-core [3, 8] outputs and undoes the batch
    permutation.
"""

import numpy as np

B, S, H = 64, 512, 1024
HID = 512
EPS = 1e-5
NCORES = 8
BPC = B // NCORES  # batches per core
NMC = 3 * BPC      # mask columns: (embedding e, batch slot b) -> e*BPC + b
MW = NMC           # mask width (cols 0:NMC of each chunk)
PADW = 32          # mask region padded so CW % 16 == 0 (DoubleRow AP rule)
CW = PADW + H      # chunk width: mask region + bert row

# Span rows >= T ship as fp8 e4m3 (DoubleRow-capable); shorter spans
# (and pron rows) ship bf16.
T_FP8 = 16
W1_SCALE = 16.0    # prescale for W1 span rows in fp8 e3m4
N_WARM = 8         # dummy matmuls to pre-warm the PE clock
RING_RATE = 0.195  # per-ring stream rate, B/ns (for arrival ordering)

TRACE = False
LAST_RESULT = None

_PROGRAM_CACHE: dict = {}

# W1 piece layout (consumption-ordered, small tail pieces). Span pieces
# slice the [128, 16, HID] e3m4 tensor (k = hc*2+e); pron pieces slice
# the [128, 8, HID] bf16 tensor (k = hc). Piece count is kept low: the
# shared 10-semaphore DMA pool serializes issue across rings when
# descriptors are too numerous.
W1S_PIECES = [(0, 8), (8, 4), (12, 2), (14, 1), (15, 1)]
W1P_PIECES = [(0, 4), (4, 2), (6, 1), (7, 1)]
E3_PIECE = 2       # chunks per e3 DMA piece (multiple of 2 for DoubleRow)

# cstAB layout (single [128, 560] bf16 const buffer; >=512B lines so the
# SDMA engines run at line rate): [0:24]=ident24, [24:36]=W2 (4x3),
# [36]=b2 (rows 0:3), row0 [37:549]=bn bias, row0 [549:557]=ones
CAB_W = 560


def _build_program(nch8: int, nchb: int, bp: int):
    import concourse.bacc as bacc
    import concourse.tile as tile
    import concourse.mybir as mybir
    from concourse.bass import MemorySpace

    f32 = mybir.dt.float32
    bf = mybir.dt.bfloat16
    e4 = mybir.dt.float8e4
    w8 = mybir.dt.float8e3
    DR = mybir.MatmulPerfMode.DoubleRow

    nc = bacc.Bacc("TRN2", target_bir_lowering=False, debug=False,
                   num_devices=NCORES)

    HC = H // 128       # 8

    e3_d = nc.dram_tensor("e3buf", [128, nch8, CW], e4,
                          kind="ExternalInput").ap()
    bf_d = nc.dram_tensor("bfbuf", [bp, nchb, CW], bf,
                          kind="ExternalInput").ap()
    w1s_d = nc.dram_tensor("w1S", [128, 16, HID], w8,
                           kind="ExternalInput").ap()
    w1p_d = nc.dram_tensor("w1P", [128, 8, HID], bf,
                           kind="ExternalInput").ap()
    cstAB_d = nc.dram_tensor("cstAB", [128, CAB_W], bf,
                             kind="ExternalInput").ap()
    # sfac replicated 128x along the free dim so the DMA moves 512B lines
    sfac_d = nc.dram_tensor("sfac", [NMC, 128], f32, kind="ExternalInput").ap()
    out_d = nc.dram_tensor("out", [3, BPC], f32, kind="ExternalOutput").ap()

    with tile.TileContext(nc) as tc:
        with (
            tc.tile_pool(name="data", bufs=1) as data,
            tc.tile_pool(name="work", bufs=1) as work,
            tc.tile_pool(name="psum_x", bufs=1, space=MemorySpace.PSUM) as psx,
            tc.tile_pool(name="psum_t", bufs=4, space=MemorySpace.PSUM) as pst,
            tc.tile_pool(name="psum_h", bufs=1, space=MemorySpace.PSUM) as psh,
        ):
            # --- PE pre-warm on memset junk (fed by the otherwise-idle
            # Vector engine, whose preamble retires earliest).
            junk = work.tile([128, 512], bf, tag="junk")
            nc.vector.memset(junk, 0.0)
            scr = psh.tile([24, 512], f32, tag="scratch")
            for _ in range(N_WARM):
                nc.tensor.matmul(scr, junk[:, 0:24], junk, start=True,
                                 stop=True)

            # --- DMA issue. ring0 = sync, ring1 = scalar (HWDGE).
            # ring1 leads with the tiny consts, then the bf16 buffer;
            # e3 pieces are balanced across rings by bytes; W1 pieces
            # follow strictly after bert, in consumption order.
            ring_eng = [nc.sync, nc.scalar]
            ring_bytes = [0.0, 0.0]

            cstAB_t = data.tile([128, CAB_W], bf, tag="cstAB")
            nc.gpsimd.dma_start(out=cstAB_t, in_=cstAB_d)
            sfac_t = data.tile([NMC, 128], f32, tag="sfac")
            nc.gpsimd.dma_start(out=sfac_t, in_=sfac_d)

            bf_t = data.tile([bp, nchb, CW], bf, tag="bfp")
            r = int(np.argmin(ring_bytes))
            ring_eng[r].dma_start(out=bf_t, in_=bf_d)
            ring_bytes[r] += bp * nchb * CW * 2

            # e3 pieces: E3_PIECE chunks each (chunk pairs stay within a
            # piece for DoubleRow), assigned greedily to the lighter
            # ring; remember arrival estimates.
            e3_ts = []
            for i, c0 in enumerate(range(0, nch8, E3_PIECE)):
                pc = min(E3_PIECE, nch8 - c0)
                t = data.tile([128, pc, CW], e4, tag=f"e3p{i}",
                              name=f"e3p{i}")
                r = int(np.argmin(ring_bytes))
                ring_eng[r].dma_start(out=t, in_=e3_d[:, c0:c0 + pc, :])
                ring_bytes[r] += pc * 128 * CW
                e3_ts.append((ring_bytes[r] / RING_RATE, pc, t))

            w1s_ts = []
            for i, (k0, kn) in enumerate(W1S_PIECES):
                t = data.tile([128, kn, HID], w8, tag=f"w1s{i}",
                              name=f"w1s{i}")
                ring_eng[0].dma_start(out=t, in_=w1s_d[:, k0:k0 + kn, :])
                w1s_ts.append((k0, kn, t))
            w1p_ts = []
            for i, (k0, kn) in enumerate(W1P_PIECES):
                t = data.tile([128, kn, HID], bf, tag=f"w1p{i}",
                              name=f"w1p{i}")
                ring_eng[1].dma_start(out=t, in_=w1p_d[:, k0:k0 + kn, :])
                w1p_ts.append((k0, kn, t))

            def w1_slice(hc, e):
                if e < 2:
                    k = hc * 2 + e
                    for k0, kn, t in w1s_ts:
                        if k0 <= k < k0 + kn:
                            return t[:, k - k0, :]
                for k0, kn, t in w1p_ts:
                    if k0 <= hc < k0 + kn:
                        return t[:, hc - k0, :]
                raise AssertionError

            # ACT table loads for Identity/Lrelu
            jact = work.tile([128, 32], bf, tag="jact")
            nc.scalar.activation(jact, junk[:, 0:32],
                                 mybir.ActivationFunctionType.Identity,
                                 scale=2.0)
            nc.scalar.activation(jact, junk[:, 0:32],
                                 mybir.ActivationFunctionType.Lrelu,
                                 alpha=0.01)

            # --- mm1: x[24, 1024] += mask.T @ bert over chunks.
            # bf16 chunks first (earliest arrival, run during the clock
            # ramp), then fp8 chunk pairs via DoubleRow in estimated
            # arrival order.
            px0 = psx.tile([NMC, 512], f32, tag="px0")
            px1 = psx.tile([NMC, 512], f32, tag="px1")

            order = sorted(range(len(e3_ts)), key=lambda i: e3_ts[i][0])

            def mm1_pairs(j, first, last_any):
                _, pc, t = e3_ts[order[j]]
                for lp in range(0, pc, 2):
                    m = t[:, lp:lp + 2, 0:MW]
                    nc.tensor.matmul(px0, m, t[:, lp:lp + 2, PADW:PADW + 512],
                                     start=first and lp == 0, stop=False,
                                     perf_mode=DR)
                    nc.tensor.matmul(px1, m, t[:, lp:lp + 2, PADW + 512:CW],
                                     start=first and lp == 0,
                                     stop=last_any and lp + 2 >= pc,
                                     perf_mode=DR)

            head = min(2, len(order))
            for j in range(head):
                mm1_pairs(j, j == 0, False)
            for c in range(nchb):
                m = bf_t[:, c, 0:MW]
                nc.tensor.matmul(px0, m, bf_t[:, c, PADW:PADW + 512],
                                 start=False, stop=False)
                nc.tensor.matmul(px1, m, bf_t[:, c, PADW + 512:CW],
                                 start=False, stop=False)
            for j in range(head, len(order)):
                mm1_pairs(j, False, j == len(order) - 1)

            # --- x: scale by 1/len (fp32) + cast bf16, split ACT/DVE
            # --- BN bias into ph (consts arrived via SWDGE ~14us)
            ph = psh.tile([BPC, HID], f32, tag="ph")
            nc.tensor.matmul(ph, cstAB_t[0:1, 549:557], cstAB_t[0:1, 37:549],
                             start=True, stop=False)

            # --- x: scale by 1/len (fp32) + cast bf16 in quarter
            # slices (ACT on px0 halves, DVE on px1 halves) so the PE
            # transposes start as soon as their slice lands
            xsb = work.tile([NMC, H], bf, tag="xsb")
            ident24 = cstAB_t[0:NMC, 0:NMC]
            xT = work.tile([128, HC, NMC], bf, tag="xT")
            for half in range(2):
                s0 = half * 256
                nc.scalar.activation(xsb[:, s0:s0 + 256], px0[:, s0:s0 + 256],
                                     mybir.ActivationFunctionType.Identity,
                                     scale=sfac_t[:, 0:1])
                nc.vector.tensor_scalar_mul(xsb[:, 512 + s0:768 + s0],
                                            px1[:, s0:s0 + 256],
                                            sfac_t[:, 0:1])
            for hc in range(HC):
                pT = pst.tile([128, NMC], bf, tag="pT")
                nc.tensor.transpose(pT, xsb[:, hc * 128:(hc + 1) * 128],
                                    ident24)
                nc.vector.tensor_copy(xT[:, hc, :], pT)
            for hc in range(HC):
                for e in range(3):
                    nc.tensor.matmul(
                        ph, xT[:, hc, e * BPC:(e + 1) * BPC], w1_slice(hc, e),
                        start=False, stop=(hc == HC - 1 and e == 2),
                    )

            # --- LeakyReLU on ACT in halves (overlaps PE transposes)
            y = work.tile([BPC, HID], bf, tag="y")
            ident8 = cstAB_t[0:BPC, 0:BPC]
            yT_ps = [pst.tile([128, BPC], bf, tag="pT", name=f"yTp{mc}")
                     for mc in range(4)]
            yT_sb = [work.tile([128, BPC], bf, tag=f"yTs{mc}", name=f"yTs{mc}")
                     for mc in range(4)]
            for half in range(2):
                nc.scalar.activation(y[:, half * 256:(half + 1) * 256],
                                     ph[:, half * 256:(half + 1) * 256],
                                     mybir.ActivationFunctionType.Lrelu,
                                     alpha=0.01)
                for mc in (2 * half, 2 * half + 1):
                    nc.tensor.transpose(
                        yT_ps[mc], y[:, mc * 128:(mc + 1) * 128], ident8)
                    nc.vector.tensor_copy(yT_sb[mc], yT_ps[mc])

            # --- mm3: out[3, 8] = W2.T @ y.T; b2 folds into the copy
            oT = psx.tile([3, BPC], f32, tag="px0")
            for mc in range(4):
                nc.tensor.matmul(oT, cstAB_t[:, 24 + 3 * mc:27 + 3 * mc],
                                 yT_sb[mc], start=(mc == 0), stop=(mc == 3))
            o_sb = work.tile([3, BPC], f32, tag="osb")
            nc.scalar.activation(o_sb, oT,
                                 mybir.ActivationFunctionType.Identity,
                                 bias=cstAB_t[0:3, 36:37])
            nc.sync.dma_start(out=out_d, in_=o_sb)

    nc.compile()
    return nc


def _pack_rows(rows, masks, nch, np_dt, part=128):
    """rows: [N, H] fp32, masks: [N, NMC] fp32 -> [part, nch, CW] np_dt,
    partition-major (packed position i -> (p=i%part, c=i//part))."""
    N = rows.shape[0]
    buf = np.zeros((nch * part, CW), dtype=np.float32)
    if N:
        buf[:N, :MW] = masks
        buf[:N, PADW:] = rows
    return np.ascontiguousarray(
        buf.reshape(nch, part, CW).transpose(1, 0, 2)).astype(np_dt)


def _prep_core_inputs(bert, offs, w1_bufs, cstAB, batch_idx,
                      nch8, nchb, bp):
    import ml_dtypes
    bf16 = ml_dtypes.bfloat16
    e4 = ml_dtypes.float8_e4m3
    f8_rows, f8_masks = [], []
    b16_rows, b16_masks = [], []
    sfac = np.ones((NMC, 1), dtype=np.float32)
    for slot, gb in enumerate(batch_idx):
        a0, a1, b0, b1_, p = (int(v) for v in offs[gb])
        spans = [(a0, a1, 0), (b0, b1_, 1)]
        long_spans = [s for s in spans if s[1] - s[0] + 1 >= T_FP8]
        short_spans = [s for s in spans if s[1] - s[0] + 1 < T_FP8]
        for (lo, hi, e) in spans:
            wsc = W1_SCALE if e < 2 else 1.0
            sfac[e * BPC + slot, 0] = 1.0 / ((hi - lo + 1) * wsc)
        if long_spans:
            lo = min(s[0] for s in long_spans)
            hi = max(s[1] for s in long_spans)
            pos = np.arange(lo, hi + 1)
            keep = np.zeros(len(pos), dtype=bool)
            m = np.zeros((len(pos), NMC), dtype=np.float32)
            for (s0, s1, e) in long_spans:
                sel = (pos >= s0) & (pos <= s1)
                keep |= sel
                m[sel, e * BPC + slot] = 1.0
            f8_rows.append(bert[gb, pos[keep]])
            f8_masks.append(m[keep])
        want = {}
        for (s0, s1, e) in short_spans:
            for r in range(s0, s1 + 1):
                want.setdefault(r, []).append(e)
        want.setdefault(p, []).append(2)
        if want:
            rs = sorted(want)
            m = np.zeros((len(rs), NMC), dtype=np.float32)
            for i, r in enumerate(rs):
                for e in want[r]:
                    m[i, e * BPC + slot] = 1.0
            b16_rows.append(bert[gb, rs])
            b16_masks.append(m)

    def cat(parts, w):
        return (np.concatenate(parts, axis=0) if parts
                else np.zeros((0, w), dtype=np.float32))

    in_map = {
        "e3buf": _pack_rows(cat(f8_rows, H), cat(f8_masks, NMC), nch8, e4),
        "bfbuf": _pack_rows(cat(b16_rows, H), cat(b16_masks, NMC), nchb, bf16,
                            part=bp),
        "cstAB": cstAB,
        "sfac": np.repeat(sfac, 128, axis=1),
    }
    in_map.update(w1_bufs)
    return in_map


def _row_counts(offs):
    """Per-batch (fp8 rows, bf16 rows) under the T_FP8 split."""
    n8 = np.zeros(B, dtype=np.int64)
    nb = np.zeros(B, dtype=np.int64)
    for gb in range(B):
        a0, a1, b0, b1_, p = (int(v) for v in offs[gb])
        spans = [(a0, a1), (b0, b1_)]
        longs = [s for s in spans if s[1] - s[0] + 1 >= T_FP8]
        shorts = [s for s in spans if s[1] - s[0] + 1 < T_FP8]
        if longs:
            lo = min(s[0] for s in longs)
            hi = max(s[1] for s in longs)
            keep = np.zeros(hi - lo + 1, dtype=bool)
            for (s0, s1) in longs:
                keep[s0 - lo:s1 - lo + 1] = True
            n8[gb] = keep.sum()
        rows = set()
        for (s0, s1) in shorts:
            rows.update(range(s0, s1 + 1))
        rows.add(p)
        nb[gb] = len(rows)
    return n8, nb


def kernel(bert_outputs, offsets, W1, b1, gamma, beta, running_mean,
           running_var, W2, b2):
    import ml_dtypes
    bf16 = ml_dtypes.bfloat16
    e3 = ml_dtypes.float8_e3m4

    bert = np.ascontiguousarray(np.asarray(bert_outputs, dtype=np.float32))
    offs = np.asarray(offsets).astype(np.int64)
    W1 = np.asarray(W1, dtype=np.float32)
    b1 = np.asarray(b1, dtype=np.float32)
    gamma = np.asarray(gamma, dtype=np.float32)
    beta = np.asarray(beta, dtype=np.float32)
    rm = np.asarray(running_mean, dtype=np.float32)
    rv = np.asarray(running_var, dtype=np.float32)
    W2 = np.asarray(W2, dtype=np.float32)
    b2 = np.asarray(b2, dtype=np.float32)

    # Fold BN eval stats: bn(xW1 + b1) = x(W1*s) + ((b1 - mean)*s + beta)
    s = gamma / np.sqrt(rv + EPS)
    bias = (b1 - rm) * s + beta
    W1s = W1 * s[None, :]
    w1ehc = W1s.reshape(3, 8, 128, HID)  # [e, hc, p, n]
    w1_bufs = {
        # [p, hc*2+e, n] for span embeddings e in {0,1}, fp8 e3m4 x16
        "w1S": np.ascontiguousarray(
            w1ehc[:2].transpose(2, 1, 0, 3).reshape(128, 16, HID)
            * W1_SCALE).astype(e3),
        # [p, hc, n] for the pron embedding, bf16
        "w1P": np.ascontiguousarray(
            w1ehc[2].transpose(1, 0, 2)).astype(bf16),
    }

    cstAB = np.zeros((128, CAB_W), dtype=np.float32)
    cstAB[:NMC, :NMC] = np.eye(NMC)
    cstAB[:, 24:36] = W2.reshape(4, 128, 3).transpose(1, 0, 2).reshape(128, 12)
    cstAB[0:3, 36] = b2
    cstAB[0, 37:549] = bias
    cstAB[0, 549:557] = 1.0
    cstAB = cstAB.astype(bf16)

    # Greedy-balance batches across cores by shipped bytes (fp8 row =
    # CW bytes, bf16 row = 2*CW), capped at BPC batches per core
    n8, nb = _row_counts(offs)
    cost = n8 + 2 * nb
    order = np.argsort(-cost, kind="stable")
    core_rows8 = np.zeros(NCORES, dtype=np.int64)
    core_rowsb = np.zeros(NCORES, dtype=np.int64)
    core_batches = [[] for _ in range(NCORES)]
    for gb in order:
        load = core_rows8 + 2 * core_rowsb
        load[np.array([len(cb) >= BPC for cb in core_batches])] = 1 << 40
        c = int(np.argmin(load))
        core_batches[c].append(int(gb))
        core_rows8[c] += n8[gb]
        core_rowsb[c] += nb[gb]
    nch8 = max(2, int((core_rows8.max() + 127) // 128))
    nch8 += nch8 % 2  # even chunk count for DoubleRow pairs
    maxb = int(core_rowsb.max())
    bp = 128  # full-partition DMA lines; narrow transfers crawl
    nchb = max(1, (maxb + 127) // 128)

    key = (nch8, nchb, bp)
    if key not in _PROGRAM_CACHE:
        _PROGRAM_CACHE[key] = _build_program(nch8, nchb, bp)
    nc = _PROGRAM_CACHE[key]

    in_maps = [
        _prep_core_inputs(bert, offs, w1_bufs, cstAB, core_batches[c],
                          nch8, nchb, bp)
        for c in range(NCORES)
    ]

    from concourse import bass_utils
    kwargs = {}
    if TRACE:
        kwargs = {"trace": True, "trace_cores": list(range(NCORES))}
    res = bass_utils.run_bass_kernel_spmd(nc, in_maps,
                                          core_ids=list(range(NCORES)),
                                          **kwargs)
    global LAST_RESULT
    LAST_RESULT = res

    out = np.empty((B, 3), dtype=np.float32)
    for c in range(NCORES):
        out[core_batches[c]] = res.results[c]["out"].T
    return out
